# revision 1
# baseline (speedup 1.0000x reference)
"""Trainium2 Bass kernel for CommunityHOP GNN message passing.

Math: for each GCN branch, reference computes relu(A_hat @ (x @ W) + b) * m.
We use associativity: A_hat @ (x @ W) = (A_hat @ x) @ W, so we aggregate raw
x once per edge set (gather rows of x by edge source, one-hot matmul into the
destination block), then apply the small 256x256 weight per 128-node block.
Branch softmax-attention masks are folded into the weights/biases on the host
(relu commutes with multiplication by a positive scalar).

Sharding: nodes (and their incoming edges) are sharded by destination across
8 cores; x is replicated. The only collective is one AllGather of the first
GCN layer's output e (needed as gather source of the second GCN layer).

Per (set, 128-dst-node block): edges sorted by dst, split into src<32768 /
src>=32768 runs (dma_gather indices are int16), padded to multiples of 128.
Each 128-edge chunk: gather G=[128e,256f] rows, build S[e,n]=v_e*(off_e==n)
on DVE, accumulate PSUM agg[n,f] += S.T @ G on PE. The aggregated block is
transposed (PE identity-matmul) to feed weight matmuls that contract over
features, producing transposed branch outputs so bias+relu are per-partition
ops on ACT; transposed outputs stream into the final classifier matmul and a
per-partition log_softmax.
"""

import math
import os
from dataclasses import dataclass, field

import numpy as np


# ---------------------------------------------------------------- config

@dataclass
class Cfg:
    n: int = 50000           # nodes
    f: int = 256             # feature dim (must be 256: 2 partition tiles)
    o: int = 40              # output classes
    hops: int = 3
    ncores: int = 8
    split: int = 32768       # int16 index split for dma_gather
    mm_f32r: bool = False    # use float32r for the hot aggregation matmuls
    gather_sub: int = 8      # if >0, cap chunks per dma_gather call

    @property
    def npc(self):
        return self.n // self.ncores

    @property
    def nblk(self):
        return (self.npc + 127) // 128


# ------------------------------------------------------------- host prep

def _prep_edges(cfg, edge_sets):
    """edge_sets: list of (src, dst) int64 (self loops NOT yet included).
    Returns (sched, idx_arr, voff_arr) where
      sched[k][b] = (c_lo, c_hi, idx_col, voff_col)   -- unified over cores
      idx_arr  [ncores, 128, idxcols] int16
      voff_arr [ncores, 128, vcols] float32  (per (k,b): CB v-cols then CB off-cols)
    """
    N, NC, NPC, NBLK, SPLIT = cfg.n, cfg.ncores, cfg.npc, cfg.nblk, cfg.split
    K = len(edge_sets)
    loops = np.arange(N, dtype=np.int64)

    per = {}   # (k,c) -> (blk, half, idx16, off, v) sorted by (blk, half)
    cnt = np.zeros((K, NC, NBLK, 2), np.int64)
    for k, (src, dst) in enumerate(edge_sets):
        src = np.concatenate([src, loops])
        dst = np.concatenate([dst, loops])
        deg = np.bincount(dst, minlength=N).astype(np.float32)
        dinv = (1.0 / np.sqrt(deg)).astype(np.float32)
        v = dinv[src] * dinv[dst]
        core = dst // NPC
        for c in range(NC):
            m = core == c
            s_c = src[m]
            d_c = dst[m] - c * NPC
            v_c = v[m]
            blk = d_c >> 7
            off = (d_c & 127).astype(np.float32)
            half = (s_c >= SPLIT).astype(np.int64)
            idx16 = (s_c - SPLIT * half).astype(np.int16)
            order = np.lexsort((half, blk))
            blk, half, idx16, off, v_c = (
                blk[order], half[order], idx16[order], off[order], v_c[order])
            per[(k, c)] = (blk, half, idx16, off, v_c)
            cnt[k, c] += np.bincount(
                blk * 2 + half, minlength=NBLK * 2).reshape(NBLK, 2)

    # unified chunk counts (max over cores), rounded up to chunks of 128
    cmax = -(-cnt.max(axis=1) // 128)          # [K, NBLK, 2]
    cb = cmax.sum(axis=2)                      # [K, NBLK]
    idx_col = np.zeros((K, NBLK), np.int64)
    voff_col = np.zeros((K, NBLK), np.int64)
    run = 0
    runv = 0
    for k in range(K):
        for b in range(NBLK):
            idx_col[k, b] = run
            voff_col[k, b] = runv
            run += 8 * cb[k, b]
            runv += 2 * cb[k, b]
    idxcols, vcols = int(run), int(runv)

    idx_arr = np.zeros((NC, 128, idxcols), np.int16)
    voff_arr = np.zeros((NC, 128, vcols), np.float32)
    for k in range(K):
        for c in range(NC):
            blk, half, idx16, off, v_c = per[(k, c)]
            # starts of each (b, half) group in the sorted arrays
            key = blk * 2 + half
            starts = np.searchsorted(key, np.arange(NBLK * 2))
            ends = np.searchsorted(key, np.arange(NBLK * 2) + 1)
            for b in range(NBLK):
                ic0 = idx_col[k, b]
                vc0 = voff_col[k, b]
                CB = cb[k, b]
                ch0 = 0
                for h in (0, 1):
                    C = int(cmax[k, b, h])
                    if C == 0:
                        continue
                    s0, e0 = starts[b * 2 + h], ends[b * 2 + h]
                    ne = e0 - s0
                    L = C * 128
                    pidx = np.zeros(L, np.int16)
                    pidx[:ne] = idx16[s0:e0]
                    poff = np.zeros(L, np.float32)
                    poff[:ne] = off[s0:e0]
                    pv = np.zeros(L, np.float32)
                    pv[:ne] = v_c[s0:e0]
                    # idx: wrapped in 16 partitions, replicated to 128
                    w = pidx.reshape(L // 16, 16).T           # [16, L/16]
                    idx_arr[c, :, ic0 + 8 * ch0: ic0 + 8 * ch0 + L // 16] = (
                        np.tile(w, (8, 1)))
                    # v / off: chunk i -> col, edge within chunk -> partition
                    voff_arr[c, :, vc0 + ch0: vc0 + ch0 + C] = (
                        pv.reshape(C, 128).T)
                    voff_arr[c, :, vc0 + CB + ch0: vc0 + CB + ch0 + C] = (
                        poff.reshape(C, 128).T)
                    ch0 += C

    sched = [[(int(cmax[k, b, 0]), int(cmax[k, b, 1]),
               int(idx_col[k, b]), int(voff_col[k, b]))
              for b in range(NBLK)] for k in range(K)]
    return sched, idx_arr, voff_arr


def _prep_all(cfg, inputs):
    """Full host-side prep. Returns (sched, in_maps)."""
    N, F, O, H = cfg.n, cfg.f, cfg.o, cfg.hops
    x = np.ascontiguousarray(np.asarray(inputs["x"], np.float32))
    ei = np.asarray(inputs["edge_index"], np.int64)
    nei = np.asarray(inputs["new_edge_indexs"], np.int64)
    att = np.asarray(inputs["att"], np.float32)

    m = np.exp(att - att.max())
    m = (m / m.sum()).astype(np.float32)

    W_mlp = np.asarray(inputs["W_mlp"], np.float32) * m[0]
    b_mlp = np.asarray(inputs["b_mlp"], np.float32) * m[0]
    We1 = np.asarray(inputs["We1"], np.float32)
    be1 = np.asarray(inputs["be1"], np.float32)
    We2 = np.asarray(inputs["We2"], np.float32) * m[1]
    be2 = np.asarray(inputs["be2"], np.float32) * m[1]
    Wh = np.asarray(inputs["Wh"], np.float32).copy()
    bh = np.asarray(inputs["bh"], np.float32).copy()
    for i in range(H):
        Wh[i] *= m[i + 1]
        bh[i] *= m[i + 1]
    Wc = np.asarray(inputs["Wc"], np.float32)
    bc = np.asarray(inputs["bc"], np.float32)

    edge_sets = [(ei[0], ei[1])] + [(nei[i, 0], nei[i, 1]) for i in range(H)]
    sched, idx_arr, voff_arr = _prep_edges(cfg, edge_sets)

    # weights, branch order for lhsT form: [e2, h0, h1, h2, mlp]
    def lhsT_tiles(W):  # [2,2,128,128]: [kt][mt] = W[128kt:.., 128mt:..]
        return W.reshape(2, 128, 2, 128).transpose(0, 2, 1, 3)

    w_lhsT = np.stack([lhsT_tiles(We2)] + [lhsT_tiles(Wh[i]) for i in range(H)]
                      + [lhsT_tiles(W_mlp)]).astype(np.float32)  # [5,2,2,128,128]
    w_rhs_e1 = We1.reshape(2, 128, F).astype(np.float32)
    ntile = (F * (H + 2)) // 128   # 10
    wcb = np.zeros((ntile + 1, 128, O), np.float32)
    wcb[:ntile] = Wc.reshape(ntile, 128, O)
    wcb[ntile, 0, :] = bc
    bias_sb = np.zeros((128, 10), np.float32)   # [p, 2*branch+half]
    for bi, bv in enumerate([be2, bh[0], bh[1], bh[2], b_mlp]):
        for h in (0, 1):
            bias_sb[:, 2 * bi + h] = bv[128 * h: 128 * h + 128]
    be1_bc = np.tile(be1[None, :], (128, 1)).astype(np.float32)
    consts = np.zeros((2, 128, 128), np.float32)
    consts[0] = np.tile(np.arange(128, dtype=np.float32)[None, :], (128, 1))
    consts[1] = np.eye(128, dtype=np.float32)

    in_maps = []
    for c in range(cfg.ncores):
        xt_own = np.ascontiguousarray(
            x[c * cfg.npc:(c + 1) * cfg.npc].T.reshape(2, 128, cfg.npc))
        in_maps.append({
            "x": x,
            "xt_own": xt_own,
            "idx": np.ascontiguousarray(idx_arr[c]),
            "voff": np.ascontiguousarray(voff_arr[c]),
            "w_lhsT": w_lhsT,
            "w_rhs_e1": w_rhs_e1,
            "wcb": wcb,
            "bias_sb": bias_sb,
            "be1_bc": be1_bc,
            "consts": consts,
        })
    return sched, in_maps


# --------------------------------------------------------- program build

def build_program(cfg, sched):
    import concourse.bass as bass
    import concourse.mybir as mybir
    import concourse.tile as tile
    from concourse import bacc, library_config
    from concourse.replica_groups import maybe_share_collective_output_space

    dt = mybir.dt
    f32 = dt.float32
    alu = mybir.AluOpType
    act_f = mybir.ActivationFunctionType

    N, F, O, H, NC = cfg.n, cfg.f, cfg.o, cfg.hops, cfg.ncores
    NPC, NBLK, SPLIT = cfg.npc, cfg.nblk, cfg.split
    K = 1 + H
    idxcols = sched[K - 1][NBLK - 1][2] + 8 * (
        sched[K - 1][NBLK - 1][0] + sched[K - 1][NBLK - 1][1])
    vcols = sched[K - 1][NBLK - 1][3] + 2 * (
        sched[K - 1][NBLK - 1][0] + sched[K - 1][NBLK - 1][1])
    gmax = max(max(s[0], s[1]) for ks in sched for s in ks)
    cbmax = max(s[0] + s[1] for ks in sched for s in ks)

    nc = bacc.Bacc("TRN2", target_bir_lowering=False, debug=False,
                   num_devices=NC)

    # dtype used for the hot aggregation matmul path (gather src, G, S)
    mdt = dt.float32r if cfg.mm_f32r else f32

    x_d = nc.dram_tensor("x", [N, F], mdt, kind="ExternalInput").ap()
    xt_d = nc.dram_tensor("xt_own", [2, 128, NPC], f32,
                          kind="ExternalInput").ap()
    idx_d = nc.dram_tensor("idx", [128, idxcols], dt.int16,
                           kind="ExternalInput").ap()
    voff_d = nc.dram_tensor("voff", [128, vcols], f32,
                            kind="ExternalInput").ap()
    wl_d = nc.dram_tensor("w_lhsT", [5, 2, 2, 128, 128], f32,
                          kind="ExternalInput").ap()
    wr_d = nc.dram_tensor("w_rhs_e1", [2, 128, F], f32,
                          kind="ExternalInput").ap()
    wcb_d = nc.dram_tensor("wcb", [11, 128, O], f32,
                           kind="ExternalInput").ap()
    bias_d = nc.dram_tensor("bias_sb", [128, 10], f32,
                            kind="ExternalInput").ap()
    be1_d = nc.dram_tensor("be1_bc", [128, F], f32,
                           kind="ExternalInput").ap()
    const_d = nc.dram_tensor("consts", [2, 128, 128], f32,
                             kind="ExternalInput").ap()
    out_d = nc.dram_tensor("out_z", [NPC, O], f32, kind="ExternalOutput").ap()

    groups = [list(range(NC))]
    ag_space = maybe_share_collective_output_space("AllGather", groups)

    with tile.TileContext(nc) as tc:
        import contextlib
        ctx = contextlib.ExitStack()
        with ctx:
            const = ctx.enter_context(tc.tile_pool(name="const", bufs=1))
            g_pool = ctx.enter_context(tc.tile_pool(name="gpool", bufs=3))
            io_pool = ctx.enter_context(tc.tile_pool(name="iopool", bufs=3))
            s_pool = ctx.enter_context(tc.tile_pool(name="spool", bufs=6))
            sb_pool = ctx.enter_context(tc.tile_pool(name="sbpool", bufs=3))
            zcls_pool = ctx.enter_context(tc.tile_pool(name="zcls", bufs=2))
            p_agg = ctx.enter_context(
                tc.tile_pool(name="pagg", bufs=2, space="PSUM"))
            p_t = ctx.enter_context(
                tc.tile_pool(name="pt", bufs=2, space="PSUM"))
            p_w = ctx.enter_context(
                tc.tile_pool(name="pw", bufs=2, space="PSUM"))
            p_c = ctx.enter_context(
                tc.tile_pool(name="pc", bufs=2, space="PSUM"))
            dram = ctx.enter_context(
                tc.tile_pool(name="dram", bufs=1, space="DRAM"))

            # ---- constants
            nc.gpsimd.load_library(library_config.mlp)
            iota_f = const.tile([128, 128], f32)
            nc.sync.dma_start(out=iota_f[:], in_=const_d[0])
            ident = const.tile([128, 128], f32)
            nc.sync.dma_start(out=ident[:], in_=const_d[1])
            ones_row = const.tile([128, 128], f32)
            nc.vector.memset(ones_row[:], 0.0)
            nc.vector.memset(ones_row[0:1, :], 1.0)

            wl = const.tile([128, 20, 128], f32)
            for g in range(20):
                b_, kt, mt = g // 4, (g // 2) % 2, g % 2
                nc.sync.dma_start(out=wl[:, g, :], in_=wl_d[b_, kt, mt])
            wr = const.tile([128, 2, F], f32)
            for kt in range(2):
                nc.sync.dma_start(out=wr[:, kt, :], in_=wr_d[kt])
            wcb = const.tile([128, 11, O], f32)
            for t in range(11):
                nc.sync.dma_start(out=wcb[:, t, :], in_=wcb_d[t])
            bias_sb = const.tile([128, 10], f32)
            nc.sync.dma_start(out=bias_sb[:], in_=bias_d[:])
            be1_bc = const.tile([128, F], f32)
            nc.sync.dma_start(out=be1_bc[:], in_=be1_d[:])

            e_loc = dram.tile([NPC, F], mdt)
            e_full = dram.tile([N, F], mdt, addr_space=ag_space)
            zt_dram = dram.tile([NBLK, 8, 128, 128], f32)

            def aggregate(k, b, lo_ap, hi_ap):
                """Gather + one-hot matmul for (set k, block b).
                Returns aggT sbuf tile [128, 2, 128] (features on partitions)."""
                c_lo, c_hi, ic0, vc0 = sched[k][b]
                CB = c_lo + c_hi
                assert CB > 0
                idx_t = io_pool.tile([128, 8 * cbmax], dt.int16, tag="idx")
                nc.sync.dma_start(out=idx_t[:, :8 * CB],
                                  in_=idx_d[:, ic0:ic0 + 8 * CB])
                voff_t = io_pool.tile([128, 2 * cbmax], f32, tag="voff")
                nc.sync.dma_start(out=voff_t[:, :2 * CB],
                                  in_=voff_d[:, vc0:vc0 + 2 * CB])
                agg_ps = p_agg.tile([128, F], f32, tag="agg")
                ch = 0
                for half, crun, src_ap in ((0, c_lo, lo_ap), (1, c_hi, hi_ap)):
                    if crun == 0:
                        continue
                    icol = 0 if half == 0 else 8 * c_lo
                    g_t = g_pool.tile([128, gmax, F], mdt, tag="G")
                    sub = cfg.gather_sub or crun
                    done = 0
                    while done < crun:
                        take = min(sub, crun - done)
                        nc.gpsimd.dma_gather(
                            g_t[:, done:done + take, :], src_ap,
                            idx_t[:, icol + 8 * done:icol + 8 * (done + take)],
                            num_idxs=take * 128, num_idxs_reg=take * 128,
                            elem_size=F, elem_step=F)
                        done += take
                    for j in range(crun):
                        s_t = s_pool.tile([128, 128], mdt, tag="S")
                        nc.vector.tensor_scalar(
                            out=s_t[:], in0=iota_f[:],
                            scalar1=voff_t[:, CB + ch:CB + ch + 1],
                            scalar2=voff_t[:, ch:ch + 1],
                            op0=alu.is_equal, op1=alu.mult)
                        nc.tensor.matmul(
                            agg_ps[:], lhsT=s_t[:], rhs=g_t[:, j, :],
                            start=(ch == 0), stop=(ch == CB - 1))
                        ch += 1
                # evacuate + transpose
                agg_sb = sb_pool.tile([128, F], f32, tag="aggsb")
                nc.scalar.copy(agg_sb[:], agg_ps[:])
                aggT = sb_pool.tile([128, 2, 128], f32, tag="aggT")
                for h in (0, 1):
                    pt_ps = p_t.tile([128, 128], f32, tag="pt")
                    nc.tensor.transpose(pt_ps[:], agg_sb[:, 128 * h:128 * (h + 1)],
                                        ident[:])
                    nc.vector.tensor_copy(aggT[:, h, :], pt_ps[:])
                return aggT

            def branch_T(rhsT, bidx, out_tiles):
                """Transposed branch: out[m] = relu(W.T @ rhs + b), m=0,1.
                rhsT: [128, 2, 128] tile; bidx: index into [e2,h0,h1,h2,mlp].
                out_tiles: list of 2 sbuf [128,128] tiles to fill."""
                for mt in (0, 1):
                    pw_ps = p_w.tile([128, F], f32, tag="pw")
                    nc.tensor.matmul(pw_ps[:, :128],
                                     lhsT=wl[:, (2 * bidx + 0) * 2 + mt, :],
                                     rhs=rhsT[:, 0, :], start=True, stop=False)
                    nc.tensor.matmul(pw_ps[:, :128],
                                     lhsT=wl[:, (2 * bidx + 1) * 2 + mt, :],
                                     rhs=rhsT[:, 1, :], start=False, stop=True)
                    nc.scalar.activation(
                        out_tiles[mt][:], pw_ps[:, :128], act_f.Relu,
                        bias=bias_sb[:, 2 * bidx + mt:2 * bidx + mt + 1],
                        scale=1.0)

            def block_rows(b):
                nr = min(128, NPC - b * 128)
                return b * 128, nr

            x_lo = x_d[0:SPLIT, :]
            x_hi = x_d[SPLIT:N, :]

            # ---------------- phase 1a: set 0 (ei) -> e rows + allgather
            for b in range(NBLK):
                r0, nr = block_rows(b)
                aggT = aggregate(0, b, x_lo, x_hi)
                pe_ps = p_w.tile([128, F], f32, tag="pw")
                nc.tensor.matmul(pe_ps[:], lhsT=aggT[:, 0, :], rhs=wr[:, 0, :],
                                 start=True, stop=False)
                nc.tensor.matmul(pe_ps[:], lhsT=aggT[:, 1, :], rhs=wr[:, 1, :],
                                 start=False, stop=True)
                e_sb = sb_pool.tile([128, F], f32, tag="esb")
                nc.vector.tensor_tensor(out=e_sb[:], in0=pe_ps[:],
                                        in1=be1_bc[:], op=alu.add)
                nc.vector.tensor_scalar(out=e_sb[:], in0=e_sb[:],
                                        scalar1=0.0, scalar2=None,
                                        op0=alu.max)
                nc.sync.dma_start(out=e_loc[r0:r0 + nr, :].bitcast(f32),
                                  in_=e_sb[:nr, :])

            nc.gpsimd.collective_compute(
                "AllGather", alu.bypass, replica_groups=groups,
                ins=[e_loc[:].opt()], outs=[e_full[:].opt()])

            # ---------------- phase 1b: hop sets 1..3 (+ mlp on last set)
            for k in range(1, K):
                for b in range(NBLK):
                    aggT = aggregate(k, b, x_lo, x_hi)
                    zt0 = sb_pool.tile([128, 128], f32, tag="zt", bufs=6)
                    zt1 = sb_pool.tile([128, 128], f32, tag="zt", bufs=6)
                    branch_T(aggT, k, [zt0, zt1])  # bidx: h0=1,h1=2,h2=3
                    slot = 2 * (k - 1)
                    nc.scalar.dma_start(out=zt_dram[b, slot], in_=zt0[:])
                    nc.scalar.dma_start(out=zt_dram[b, slot + 1], in_=zt1[:])
                    if k == K - 1:
                        r0, nr = block_rows(b)
                        xtt = sb_pool.tile([128, 2, 128], f32, tag="xtt")
                        if nr < 128:
                            nc.vector.memset(xtt[:], 0.0)
                        for kt in (0, 1):
                            nc.sync.dma_start(out=xtt[:, kt, :nr],
                                              in_=xt_d[kt, :, r0:r0 + nr])
                        zm0 = sb_pool.tile([128, 128], f32, tag="zt", bufs=6)
                        zm1 = sb_pool.tile([128, 128], f32, tag="zt", bufs=6)
                        branch_T(xtt, 4, [zm0, zm1])
                        nc.scalar.dma_start(out=zt_dram[b, 6], in_=zm0[:])
                        nc.scalar.dma_start(out=zt_dram[b, 7], in_=zm1[:])

            # ---------------- phase 2: e2 branch + classifier + log_softmax
            e_lo = e_full[0:SPLIT, :]
            e_hi = e_full[SPLIT:N, :]
            for b in range(NBLK):
                r0, nr = block_rows(b)
                aggT2 = aggregate(0, b, e_lo, e_hi)
                e2t0 = zcls_pool.tile([128, 128], f32, tag="zcls", bufs=12)
                e2t1 = zcls_pool.tile([128, 128], f32, tag="zcls", bufs=12)
                branch_T(aggT2, 0, [e2t0, e2t1])
                zts = []
                for t in range(8):
                    zz = zcls_pool.tile([128, 128], f32, tag="zcls", bufs=12)
                    nc.sync.dma_start(out=zz[:], in_=zt_dram[b, t])
                    zts.append(zz)
                # z tile order: h0(0,1) h1(2,3) h2(4,5) e2(6,7) mlp(8,9)
                order = [zts[0], zts[1], zts[2], zts[3], zts[4], zts[5],
                         e2t0, e2t1, zts[6], zts[7]]
                pc_ps = p_c.tile([128, O], f32, tag="pcls")
                for t in range(10):
                    nc.tensor.matmul(pc_ps[:], lhsT=order[t][:],
                                     rhs=wcb[:, t, :],
                                     start=(t == 0), stop=False)
                nc.tensor.matmul(pc_ps[:], lhsT=ones_row[:], rhs=wcb[:, 10, :],
                                 start=False, stop=True)
                mx = s_pool.tile([128, 1], f32, tag="mx")
                nc.vector.tensor_reduce(mx[:], pc_ps[:],
                                        axis=mybir.AxisListType.X, op=alu.max)
                tt = s_pool.tile([128, O], f32, tag="tt")
                nc.vector.tensor_scalar(out=tt[:], in0=pc_ps[:],
                                        scalar1=mx[:, 0:1], scalar2=None,
                                        op0=alu.subtract)
                ex = s_pool.tile([128, O], f32, tag="ex")
                se = s_pool.tile([128, 1], f32, tag="se")
                nc.scalar.activation(ex[:], tt[:], act_f.Exp,
                                     accum_out=se[:])
                lse = s_pool.tile([128, 1], f32, tag="lse")
                nc.scalar.activation(lse[:], se[:], act_f.Ln)
                ot = s_pool.tile([128, O], f32, tag="ot")
                nc.vector.tensor_scalar(out=ot[:], in0=tt[:],
                                        scalar1=lse[:, 0:1], scalar2=None,
                                        op0=alu.subtract)
                nc.sync.dma_start(out=out_d[r0:r0 + nr, :], in_=ot[:nr, :])

    nc.compile()
    return nc


# ------------------------------------------------------------------ main

def _run(cfg, inputs, trace=False):
    from concourse.bass_utils import run_bass_kernel_spmd

    sched, in_maps = _prep_all(cfg, inputs)
    nc = build_program(cfg, sched)
    res = run_bass_kernel_spmd(nc, in_maps, list(range(cfg.ncores)),
                               trace=trace)
    out = np.concatenate([res.results[c]["out_z"]
                          for c in range(cfg.ncores)], axis=0)
    return out, res


def kernel(**inputs) -> np.ndarray:
    cfg = Cfg()
    out, _ = _run(cfg, inputs, trace=False)
    return out

